# revision 22
# baseline (speedup 1.0000x reference)
"""BitLinear (ternary-weight + 8-bit-activation quantized matmul) on 8 TRN2 cores.

Strategy: data-parallel over tokens. Each core gets 2048 of the 16384 tokens
plus the full weight matrix, computes the whole BitLinear forward for its
token shard on device, and the host concatenates the shards.

Math (must match the jax reference):
  w_scale = max(mean(|W|), 1e-6)                       (scalar)
  w_q     = clip(round(W / w_scale), -1, 1)            (ternary)
  a       = clip(max_i |x|, 1e-8, inf)                 (per token)
  x_q     = clip(round(x * 127 / a), -127, 127)        (8-bit ints)
  y       = (x_q @ w_q^T) * w_scale * a / 127

All rounding is done with the fp32 magic-number trick (v + 1.5*2^23 - 1.5*2^23
is round-to-nearest-even), so device results bit-match jnp.round. x_q (ints
<= 127) and w_q ({-1,0,1}) are exact in bf16 and products accumulate exactly
in fp32 PSUM (|y_int| < 2^24), so the bf16 TensorE matmul is exact.

Schedule (v2): single fused W pass -- the abs-mean scan stages the first
KST W tiles in SBUF so quantization starts the moment w_scale is known,
with no second read on the critical path. The GEMM opens with a 4x4
(token-tile x output-quarter) block so the first matmuls only need the
first quarter of quantized W; the quantize stream races the opening
block instead of the first 14us of GEMM. x-side prep never depends on
w_scale (the per-token output scale is folded in at PSUM-drain time), so
token tiles 0..3 are prepped during the quantize window. Engine split:
scalar = quantize IDENTITYs, DVE = clips/small chains/PSUM drains,
GpSimd = abs-max + x transposes, Sync = W transposes, sync ring = bulk
DMA.
"""

from contextlib import ExitStack

import numpy as np

import concourse.bass as bass
import concourse.tile as tile
from concourse import bacc, bass_isa, mybir
from concourse.bass import ds, ts
from concourse.bass_utils import run_bass_kernel_spmd

F32 = mybir.dt.float32
BF16 = mybir.dt.bfloat16
AF = mybir.ActivationFunctionType
OP = mybir.AluOpType
AX = mybir.AxisListType

B, S, D_IN, D_OUT = 4, 4096, 2048, 2048
N_CORES = 8
TOK = B * S                # 16384 tokens
TPC = TOK // N_CORES       # 2048 tokens per core
NT = TPC // 128            # 16 token tiles per core
NJ = D_OUT // 128          # 16 weight row tiles
NI = D_IN // 128           # 16 contraction (k) blocks
NO = D_OUT // 512          # 4 output column blocks
CM = 12582912.0            # 1.5 * 2^23: fp32 RNE rounding magic
QMAX = 127.0

KNOBS = {
    "kst": 8,            # W tiles staged in SBUF during the abs-mean scan
    "open": 4,           # token tiles in the opening (t, no) block
    "pref": 4,           # x tiles prepped before the main loop
    "ldx_bufs": 2,       # f32 x load pool
    "xqt_bufs": 4,       # transposed x tiles in flight
    "wq_bufs": 4,        # quantized W staging (pre-transpose)
    "xq_bufs": 2,
    "ys_bufs": 3,
    "wtp_eng": "s",      # W transpose engine: s=scalar(Act ring), y=sync
    "xtp_eng": "s",      # x transpose engine
    "wq_eng": "v",       # W quantize scale+round engine
    "xmax_eng": "v",     # x abs-max engine: v or g
    "xq_eng0": "vvvv",   # x quantize engine per prefetched tile
    "xq_engN": "v",      # x quantize engine steady state
    "xsm_eng0": "v",     # small-chain engine for prefetched tiles
    "xadd_eng0": "vvvv",  # -CM add engine per prefetched tile
    "p1_stage_eng": "s",  # staged-tile abs engine (s = scalar via scratch)
    # full load-ring order: pN = pass-1 W tile N, xN = x tile N,
    # rN = re-read W tile KST+N. All pN tokens must precede the rws chain.
    "ring_order": (
        [f"p{j}" for j in range(8, 16)]       # destructive (re-read later)
        + ["p0", "p1", "p2", "p3", "x0", "p4", "x1", "p5", "x2", "p6",
           "x3", "p7"]                        # staged + x prefetch
        + [f"r{i}" for i in range(8)]         # re-reads of p8..15
    ),
    # startup issue program (see _emit): wN = IDENT+clip, uN = W transpose
    # trigger, XN = x chain compute, TN = x transpose trigger,
    # rN = re-read load of W tile KST+N
    "startup_prog": ["X0", "X1", "X2", "X3",
                     "w0", "u0", "w1", "u1", "w2", "u2", "w3", "u3", "T0",
                     "w4", "u4", "T1", "w5", "u5", "w6", "u6", "T2",
                     "w7", "u7", "T3", "w8", "u8", "w9", "u9",
                     "w10", "u10", "w11", "u11", "w12", "u12",
                     "w13", "u13", "w14", "u14", "w15", "u15"],
    "slot_ms": 0.003,    # designed time per load slot in pass 1
    "rws_ms": 0.050,     # designed time of the w_scale reduce chain
    # designed times for startup_prog tokens (ms); tokens absent = no pin
    "prog_ms": (
        {f"w{j}": 0.052 + 0.002 * j for j in range(16)}
        | {f"u{j}": 0.0565 + 0.002 * j for j in range(16)}
        | {f"X{t}": 0.030 + 0.006 * t for t in range(4)}
        | {f"T{t}": 0.054 + 0.004 * t for t in range(4)}
        | {f"r{i}": 0.058 + 0.003 * i for i in range(8)}
    ),
}

_CACHE = {}


def _emit(tc: tile.TileContext, x_d: bass.AP, w_d: bass.AP, y_d: bass.AP):
    nc = tc.nc
    KST = KNOBS["kst"]
    OPEN = KNOBS["open"]
    PREF = KNOBS["pref"]

    def eng(c):
        return {"s": nc.scalar, "v": nc.vector, "g": nc.gpsimd, "y": nc.sync}[c]

    with ExitStack() as ctx:
        ld = ctx.enter_context(tc.tile_pool(name="ld", bufs=2))
        ldx = ctx.enter_context(tc.tile_pool(name="ldx", bufs=KNOBS["ldx_bufs"]))
        wstp = ctx.enter_context(tc.tile_pool(name="wstp", bufs=1))
        wqp = ctx.enter_context(tc.tile_pool(name="wqp", bufs=KNOBS["wq_bufs"]))
        xqp = ctx.enter_context(tc.tile_pool(name="xqp", bufs=KNOBS["xq_bufs"]))
        xqtp = ctx.enter_context(tc.tile_pool(name="xqtp", bufs=KNOBS["xqt_bufs"]))
        wqtp = ctx.enter_context(tc.tile_pool(name="wqtp", bufs=1))
        ysp = ctx.enter_context(tc.tile_pool(name="ysp", bufs=KNOBS["ys_bufs"]))
        stats = ctx.enter_context(tc.tile_pool(name="stats", bufs=6))
        consts = ctx.enter_context(tc.tile_pool(name="consts", bufs=1))
        psum = ctx.enter_context(
            tc.tile_pool(name="psum", bufs=8, space=bass.MemorySpace.PSUM)
        )

        cpos = consts.tile([128, 1], F32, tag="cpos")
        nc.vector.memset(cpos, CM)
        czero = consts.tile([128, 1], F32, tag="czero")
        nc.vector.memset(czero, 0.0)
        # per-token-tile stats, one column per tile: a (clipped absmax) and
        # sout (a * w_scale / 127, filled in lazily once w_scale exists)
        amat = consts.tile([128, NT], F32, tag="amat")
        soutmat = consts.tile([128, NT], F32, tag="soutmat")

        # wqT[no][i_in, jq, i_blk, o_in] = w_q[(no*4+jq)*128 + o_in, i_blk*128 + i_in]
        wqT = [
            wqtp.tile([128, NJ // NO, NI, 128], BF16, tag=f"wqT{no}", name=f"wqT{no}")
            for no in range(NO)
        ]
        # scratch for non-destructive Abs: alias the last wqT buffer, which is
        # not written until long after pass 1 (f32 view of its first 8KB)
        abs_scratch = wqT[NO - 1].bitcast(F32)[:, 0:2, :, :]  # [128,2,16,64] = 2048 f32

        # ---- fused pass 1: abs-sum of W + stage the first KST tiles ----
        # Tiles j >= KST (the LAST output quarters, whose GEMM deadline is
        # late) go first in the DMA stream: destructive in-place Abs on the
        # scalar engine (accum_out row-sum); data is discarded and re-read
        # later. Tiles j < KST (quarters 0..) land last and stay resident, so
        # quantization starts at scalar-engine pace the moment w_scale is
        # known -- no DMA on the quarter-0 critical path.
        wsums = consts.tile([128, NJ], F32, tag="wsums")
        wst_tiles = {}

        def w_p1(jj):
            if jj < KST:
                wt = wstp.tile([128, D_IN], F32, tag=f"wst{jj}", name=f"wst{jj}")
                nc.sync.dma_start(wt, w_d[ts(jj, 128), :])
                if KNOBS["p1_stage_eng"] == "s":
                    nc.scalar.activation(
                        abs_scratch, wt, AF.Abs, bias=czero,
                        accum_out=wsums[:, ds(jj, 1)],
                    )
                else:
                    eng(KNOBS["p1_stage_eng"]).reduce_sum(
                        wsums[:, ds(jj, 1)], wt, axis=AX.X,
                        apply_absolute_value=True,
                    )
                wst_tiles[jj] = wt
            else:
                wt = ld.tile([128, D_IN], F32, tag="ld", name=f"wp1_{jj}")
                nc.sync.dma_start(wt, w_d[ts(jj, 128), :])
                nc.scalar.activation(
                    wt, wt, AF.Abs, bias=czero, accum_out=wsums[:, ds(jj, 1)]
                )

        # ---- x prep: load, per-token scale, quantize, transpose ----
        # Entirely independent of w_scale (the output scale a*ws/127 is
        # applied at PSUM drain), so prefetched tiles run under pass 1.
        def x_load(t):
            xt = ldx.tile([128, D_IN], F32, tag="ldx", name=f"xt{t}")
            nc.sync.dma_start(xt, x_d[ts(t, 128), :])
            return xt

        def x_chain(t, xt, qe, sme):
            eng(KNOBS["xmax_eng"]).reduce_max(
                amat[:, ds(t, 1)], xt, axis=AX.X, apply_absolute_value=True
            )
            a = amat[:, ds(t, 1)]
            sm = eng(sme)
            sm.tensor_scalar(a, a, 1e-8, None, OP.max)
            r0 = stats.tile([128, 1], F32, tag="xr0", name=f"xr0{t}")
            sm.reciprocal(r0, a)
            ntt = stats.tile([128, 1], F32, tag="xntt", name=f"xntt{t}")
            sm.tensor_mul(ntt, a, r0)
            sm.tensor_scalar(ntt, ntt, -1.0, 2.0, OP.mult, OP.add)
            s = stats.tile([128, 1], F32, tag="xs", name=f"xs{t}")
            sm.tensor_mul(s, r0, ntt)
            sm.tensor_scalar(s, s, QMAX, None, OP.mult)  # 127/a

            # t1 = x*s + CM in place (fp32 add at ulp=1 == RNE round)
            if qe == "s":
                nc.scalar.activation(xt, xt, AF.Identity, bias=cpos, scale=s)
            else:
                eng(qe).tensor_scalar(xt, xt, s, CM, OP.mult, OP.add)
            xq = xqp.tile([128, D_IN], BF16, tag="xq", name=f"xq{t}")
            ae = KNOBS["xadd_eng0"][t] if t < PREF else "v"
            eng(ae).tensor_scalar(xq, xt, -CM, None, OP.add)
            return xq

        def x_tpose_x(t, xq):
            xqT = xqtp.tile([128, NI, 128], BF16, tag="xqT", name=f"xqT{t}")
            eng(KNOBS["xtp_eng"]).dma_start(xqT, xq, transpose=True)
            return xqT

        def x_prep(t, qe, sme="v"):
            return x_tpose_x(t, x_chain(t, x_load(t), qe, sme))

        wre_tiles = {}
        xts = {}

        def w_reload(jj):
            # recycle the staged-tile buffer freed by quantize of jj-KST
            wt = wstp.tile([128, D_IN], F32, tag=f"wst{jj % KST}", name=f"wre{jj}")
            nc.sync.dma_start(wt, w_d[ts(jj, 128), :])
            wre_tiles[jj] = wt

        ring = list(KNOBS["ring_order"])
        # pass-1 tokens (pN) must all precede the rws chain; emit them (plus
        # any interleaved x loads) now, remember the rest for after.
        # Each load slot gets a designed schedule time (tile_wait_until) so
        # the static scheduler reproduces the intended pipeline.
        SLOT = KNOBS["slot_ms"]
        last_p = max(i for i, tok in enumerate(ring) if tok[0] == "p")
        for i, tok in enumerate(ring[: last_p + 1]):
            kind, idx = tok[0], int(tok[1:])
            with tc.tile_wait_until(i * SLOT):
                if kind == "p":
                    w_p1(idx)
                elif kind == "x":
                    xts[idx] = x_load(idx)
                else:
                    w_reload(KST + idx)
        ring_rest = ring[last_p + 1:]

        wsum_p = stats.tile([128, 1], F32, tag="wsp")
        ctx.enter_context(tc.tile_wait_until(KNOBS["rws_ms"]))
        nc.vector.reduce_sum(wsum_p, wsums, axis=AX.X)
        wsum_all = stats.tile([128, 1], F32, tag="wsa")
        nc.gpsimd.partition_all_reduce(wsum_all, wsum_p, 128, bass_isa.ReduceOp.add)
        # w_scale = max(sum / (O*I), 1e-6)
        wscale = consts.tile([128, 1], F32, tag="wscale")
        nc.vector.tensor_scalar(
            wscale, wsum_all, 1.0 / (D_OUT * D_IN), 1e-6, OP.mult, OP.max
        )
        # rws ~= 1/w_scale with one Newton refinement
        r0 = stats.tile([128, 1], F32, tag="wr0")
        nc.vector.reciprocal(r0, wscale)
        ntt = stats.tile([128, 1], F32, tag="wntt")
        nc.vector.tensor_mul(ntt, wscale, r0)
        nc.vector.tensor_scalar(ntt, ntt, -1.0, 2.0, OP.mult, OP.add)
        rws = consts.tile([128, 1], F32, tag="rws")
        nc.vector.tensor_mul(rws, r0, ntt)
        ws127 = consts.tile([128, 1], F32, tag="ws127")
        nc.vector.tensor_scalar(ws127, wscale, 1.0 / QMAX, None, OP.mult)


        wqs = {}

        def w_quant(j):
            src = wst_tiles[j] if j < KST else wre_tiles[j]
            # t1 = W*rws + CM in place
            if KNOBS["wq_eng"] == "s":
                nc.scalar.activation(src, src, AF.Identity, bias=cpos, scale=rws)
            else:
                eng(KNOBS["wq_eng"]).tensor_scalar(src, src, rws, CM, OP.mult, OP.add)
            wq = wqp.tile(
                [128, D_IN], BF16, tag=f"wqn{j % KNOBS['wq_bufs']}",
                name=f"wq{j}", bufs=1,
            )
            # (t1 - CM) min 1 -> bf16 (exact small ints), then max -1 in bf16
            nc.vector.tensor_scalar(wq, src, -CM, 1.0, OP.add, OP.min)
            nc.vector.tensor_scalar(wq, wq, -1.0, None, OP.max)
            wqs[j] = wq

        def w_tpose(j):
            eng(KNOBS["wtp_eng"]).dma_start(
                wqT[j // 4][:, j % 4, :, :], wqs[j], transpose=True
            )

        for tok in ring_rest:
            kind, idx = tok[0], int(tok[1:])
            if kind == "x":
                xts[idx] = x_load(idx)
            else:
                w_reload(KST + idx)

        # startup issue program: wN = quantize W tile N, XN = x-chain compute
        # for prefetched tile N, TN = its transpose issue. Token order tunes
        # the in-order per-engine queues (scalar IDENTs, DVE clips, ACT-ring
        # transpose issues) against the quarter deadlines of the opening
        # GEMM block.
        xqTs = {}
        xqs = {}
        for tok in KNOBS["startup_prog"]:
            kind, idx = tok[0], int(tok[1:])
            ms = KNOBS["prog_ms"].get(tok, None)
            with tc.tile_wait_until(ms if ms is not None else 0,
                                    enable=ms is not None):
                if kind == "w":
                    w_quant(idx)
                elif kind == "u":
                    w_tpose(idx)
                elif kind == "r":
                    w_reload(KST + idx)
                elif kind == "X":
                    xqs[idx] = x_chain(
                        idx, xts[idx], KNOBS["xq_eng0"][idx], KNOBS["xsm_eng0"]
                    )
                else:
                    xqTs[idx] = x_tpose_x(idx, xqs[idx])

        # ---- GEMM ----
        # opening block: (t, no) pairs column-major over t=0..OPEN-1 so the
        # first matmuls need only wqT quarter 0; remaining quarters stream in
        # behind. Then plain token-major for the rest.
        pairs = [(t, no) for no in range(NO) for t in range(OPEN)]
        pairs += [(t, no) for t in range(OPEN, NT) for no in range(NO)]

        prep_next = PREF
        sout_done = set()
        for i, (t, no) in enumerate(pairs):
            # software-pipelined x prep: one tile every 4 pairs
            if i % NO == 0 and prep_next < NT:
                xqTs[prep_next] = x_prep(prep_next, KNOBS["xq_engN"])
                prep_next += 1

            if t not in sout_done:
                nc.vector.tensor_mul(
                    soutmat[:, ds(t, 1)], amat[:, ds(t, 1)], ws127
                )
                sout_done.add(t)

            xqT = xqTs[t]
            ps = psum.tile([128, 512], F32, tag="ps")
            for b in range(NI):
                nc.tensor.matmul(
                    ps,
                    xqT[:, b, :],
                    wqT[no][:, :, b, :],
                    start=(b == 0),
                    stop=(b == NI - 1),
                )
            yt = ysp.tile([128, 512], F32, tag="ys")
            nc.vector.tensor_scalar(yt, ps, soutmat[:, ds(t, 1)], None, OP.mult)
            nc.sync.dma_start(y_d[ts(t, 128), ts(no, 512)], yt)


def _build():
    key = tuple(sorted((k, str(v)) for k, v in KNOBS.items()))
    if key in _CACHE:
        return _CACHE[key]
    nc = bacc.Bacc(
        "TRN2", target_bir_lowering=False, debug=False, num_devices=N_CORES
    )
    x_d = nc.dram_tensor("x", [TPC, D_IN], F32, kind="ExternalInput").ap()
    w_d = nc.dram_tensor("w", [D_OUT, D_IN], F32, kind="ExternalInput").ap()
    y_d = nc.dram_tensor("y", [TPC, D_OUT], F32, kind="ExternalOutput").ap()
    with tile.TileContext(nc) as tc:
        _emit(tc, x_d, w_d, y_d)
    nc.compile()
    _CACHE[key] = nc
    return nc


_last_result = None  # BassKernelResults of the most recent run (for profiling)


def kernel(x: np.ndarray, weight: np.ndarray, trace: bool = False) -> np.ndarray:
    global _last_result
    nc = _build()
    xf = np.ascontiguousarray(x.reshape(TOK, D_IN), dtype=np.float32)
    wf = np.ascontiguousarray(weight, dtype=np.float32)
    in_maps = [
        {"x": xf[c * TPC:(c + 1) * TPC], "w": wf}
        for c in range(N_CORES)
    ]
    res = run_bass_kernel_spmd(nc, in_maps, list(range(N_CORES)), trace=trace)
    _last_result = res
    y = np.concatenate([res.results[c]["y"] for c in range(N_CORES)], axis=0)
    return y.reshape(B, S, D_OUT)
